# revision 40
# baseline (speedup 1.0000x reference)
"""Bass/Trainium2 kernel for nn_BysMamba (bidirectional + stacked Mamba LM).

Sharding: DP2 x TP4. Cores 0-3 process sample 0, cores 4-7 sample 1 (the two
batch samples are independent end-to-end). Within each 4-core group the
d_inner=944 dim is tensor-parallel (236 channels/core, two 118-row tiles).
Each core keeps an fp32 master of its 118-row slice of the residual stream in
SBUF; per layer the out_proj partials are combined with ReduceScatter (landing
exactly on the owner's slice) and the updated residual is AllGather'd in bf16
for the next in_proj. The selective scan exploits the S4D init A[c,n] ~
-(n+1): most decay factors ag_n = exp(A_n*delta) come from scalar-engine Exps
with the exact per-channel A column; ag for n+1 in {4,8,12} is a DVE squaring
of the retained half-power. Scans run on GpSimd, the elementwise muls on DVE,
so the three engines (plus PE for the n-reduction matmuls) run concurrently.
"""
import sys
sys.path.insert(0, '/opt/trn_rl_repo')

import numpy as np
import ml_dtypes

import concourse.bass as bass
from concourse import bacc
import concourse.mybir as mybir
import concourse.tile as tile
from concourse.masks import make_identity
from concourse.bass_utils import run_bass_kernel_spmd

F32 = mybir.dt.float32
BF16 = mybir.dt.bfloat16
AF = mybir.ActivationFunctionType
OP = mybir.AluOpType

V = 472
DIM = 472
ED = 944
NS = 16
KC = 4
R = 30
DEPTH = 8
B = 2

NCORES = 8
TP = 4                        # cores per group (one group per sample)
CT = 118                      # partition tile
NCT = 2                       # channel tiles per core
EC4 = CT * NCT                # 236 channels per core
MT = 118                      # DIM row tile (DIM = 4*MT)
XR = R + 2 * NS               # 62 xproj rows

SETS = ['in'] + [f'l{i}' for i in range(DEPTH)] + ['out']
GROUPS = [[0, 1, 2, 3], [4, 5, 6, 7]]



def _bf(x):
    return np.ascontiguousarray(np.asarray(x, np.float32).astype(ml_dtypes.bfloat16))


def _f32(x):
    return np.ascontiguousarray(np.asarray(x, np.float32))


def prep_core_inputs(core, inputs, L):
    g, rank = divmod(core, TP)
    e0 = rank * EC4
    e1 = e0 + EC4
    d = {}
    x = np.asarray(inputs['x'], np.float32)[g]        # (L, 3, 3)
    d['x_rhs'] = _bf(x.reshape(L, 9).T)               # (9, L)
    pw = np.asarray(inputs['patch_w'], np.float32)[:, 0].reshape(V, 9)
    d['patch_lhsT'] = _bf(pw.T)                       # (9, DIM)
    pb = np.asarray(inputs['patch_b'], np.float32)
    d['patch_b'] = _f32(pb.reshape(4, MT, 1))
    d['patch_own_lhsT'] = _bf(pw.T[:, rank * MT:(rank + 1) * MT])
    d['patch_own_b'] = _f32(pb[rank * MT:(rank + 1) * MT].reshape(MT, 1))
    lm = np.asarray(inputs['lm_head_w'], np.float32)[rank * MT:(rank + 1) * MT]
    d['lm_lhsT'] = _bf(lm.T.reshape(4, MT, MT))       # (4, MT, MT)
    for s in SETS:
        if s == 'in':
            gfn = lambda n: np.asarray(inputs[f'in_{n}'], np.float32)
        elif s == 'out':
            gfn = lambda n: np.asarray(inputs[f'out_{n}'], np.float32)
        else:
            li = int(s[1:])
            gfn = lambda n, li=li: np.asarray(inputs[f'lay_{n}'], np.float32)[li]
        ip = gfn('inproj_w')
        d[f'{s}_wxi'] = _bf(ip[e0:e1].T.reshape(4, MT, EC4))
        d[f'{s}_wz'] = _bf(ip[ED + e0:ED + e1].T.reshape(4, MT, EC4))
        cw = gfn('conv_w')[e0:e1, 0]                  # (EC4, KC)
        diag = np.zeros((NCT, KC, CT, CT), np.float32)
        idx = np.arange(CT)
        for ct in range(NCT):
            for k in range(KC):
                diag[ct, k, idx, idx] = cw[ct * CT:(ct + 1) * CT, k]
        d[f'{s}_conv'] = _bf(diag)
        d[f'{s}_convb'] = _f32(gfn('conv_b')[e0:e1].reshape(NCT, CT, 1))
        d[f'{s}_xp'] = _bf(gfn('xproj_w')[:, e0:e1].T.reshape(NCT, CT, XR))
        d[f'{s}_dt'] = _bf(gfn('dt_w')[e0:e1].T)      # (R, EC4)
        d[f'{s}_dtb'] = _f32(gfn('dt_b')[e0:e1].reshape(NCT, CT, 1))
        d[f'{s}_A'] = _f32(-np.exp(gfn('Alog')[e0:e1]).reshape(NCT, CT, NS))
        d[f'{s}_D'] = _f32(gfn('D')[e0:e1].reshape(NCT, CT, 1))
        d[f'{s}_op'] = _bf(gfn('outproj_w')[:, e0:e1].T.reshape(NCT, CT, DIM))
    return d


WNAMES3 = {
    'wxi': ((4, MT, EC4), BF16), 'wz': ((4, MT, EC4), BF16),
    'convb': ((NCT, CT, 1), F32), 'xp': ((NCT, CT, XR), BF16),
    'dtb': ((NCT, CT, 1), F32), 'A': ((NCT, CT, NS), F32),
    'D': ((NCT, CT, 1), F32), 'op': ((NCT, CT, DIM), BF16),
}


class Ctx:
    pass


def build_kernel(L, repeat=1):
    nt = 512
    jts = L // nt

    nc = bacc.Bacc(num_devices=NCORES)
    din = {}

    def dram_in(name, shape, dt):
        din[name] = nc.dram_tensor(name, list(shape), dt, kind="ExternalInput")

    dram_in('x_rhs', (9, L), BF16)
    dram_in('patch_lhsT', (9, DIM), BF16)
    dram_in('patch_b', (4, MT, 1), F32)
    dram_in('patch_own_lhsT', (9, MT), BF16)
    dram_in('patch_own_b', (MT, 1), F32)
    dram_in('lm_lhsT', (4, MT, MT), BF16)
    for s in SETS:
        for w, (shp, dt) in WNAMES3.items():
            dram_in(f'{s}_{w}', shp, dt)
        dram_in(f'{s}_conv', (NCT, KC, CT, CT), BF16)
        dram_in(f'{s}_dt', (R, EC4), BF16)
    out_t = nc.dram_tensor('out', [MT, L], F32, kind="ExternalOutput")

    c = Ctx()
    c.nc, c.din, c.out_t = nc, din, out_t
    c.L, c.nt, c.jts = L, nt, jts

    with tile.TileContext(nc) as tc:
        c.tc = tc
        with (
            tc.tile_pool(name="wp", bufs=1) as wp,        # persistent
            tc.tile_pool(name="lw", bufs=2) as lw,        # streamed weights
            tc.tile_pool(name="hp", bufs=1) as hp,        # big staging tiles
            tc.tile_pool(name="ap", bufs=1) as ap_,       # per-layer activations
            tc.tile_pool(name="sp", bufs=2) as sp,        # scan transients
            tc.tile_pool(name="pp", bufs=2, space="PSUM") as pp,
            tc.tile_pool(name="yp", bufs=1, space="PSUM") as yp,
            tc.tile_pool(name="dp", bufs=1, space="DRAM") as dp,
        ):
            c.wp, c.lw, c.hp, c.ap, c.sp, c.pp, c.yp, c.dp = \
                wp, lw, hp, ap_, sp, pp, yp, dp

            ident = wp.tile([CT, CT], BF16, tag="ident")
            make_identity(nc, ident[:])
            c.ident = ident

            c.patch_lhsT = wp.tile([9, DIM], BF16, tag="patch_lhsT")
            nc.sync.dma_start(c.patch_lhsT[:], din['patch_lhsT'][:])
            c.patch_b = wp.tile([MT, 4, 1], F32, tag="patch_b")
            nc.sync.dma_start(c.patch_b[:], din['patch_b'][:].rearrange("k m o -> m k o"))
            c.patch_own_lhsT = wp.tile([9, MT], BF16, tag="patch_own_lhsT")
            nc.sync.dma_start(c.patch_own_lhsT[:], din['patch_own_lhsT'][:])
            c.patch_own_b = wp.tile([MT, 1], F32, tag="patch_own_b")
            nc.sync.dma_start(c.patch_own_b[:], din['patch_own_b'][:])
            c.lm_lhsT = wp.tile([MT, 4, MT], BF16, tag="lm_lhsT")
            nc.sync.dma_start(c.lm_lhsT[:], din['lm_lhsT'][:].rearrange("k m e -> m k e"))

            # fp32 residual master for this core's 118-row slice
            c.h_master = wp.tile([MT, L], F32, tag="h_master")

            c.hfull = dp.tile([DIM, L], BF16, tag="hfull")
            c.ccin = dp.tile([2 * XR, L], BF16, tag="ccin")
            c.ccout = dp.tile([2 * XR, L], BF16, tag="ccout")
            c.opin = dp.tile([DIM, L], BF16, tag="opin")
            c.rsout = dp.tile([MT, L], BF16, tag="rsout")
            c.agin = dp.tile([MT, L], BF16, tag="agin")

            if repeat == 1:
                build_body(c)
            else:
                with tc.For_i(0, repeat, 1):
                    build_body(c)
    nc.compile()
    return nc


def load_weights(c, s):
    """Stream layer-set s weights into double-buffered SBUF tiles."""
    nc = c.nc
    W = {}
    for w, (shp, dt) in WNAMES3.items():
        t = c.din[f'{s}_{w}']
        wt = c.lw.tile([shp[1], shp[0], shp[2]], dt, tag=f"w_{w}", name=f"w_{w}")
        nc.sync.dma_start(wt[:], t[:].rearrange("k m e -> m k e"))
        W[w] = wt
    t = c.din[f'{s}_conv']
    wt = c.lw.tile([CT, NCT, KC, CT], BF16, tag="w_conv", name="w_conv")
    nc.sync.dma_start(wt[:], t[:].rearrange("c k m e -> m c k e"))
    W['conv'] = wt
    t = c.din[f'{s}_dt']
    wt = c.lw.tile([R, EC4], BF16, tag="w_dt", name="w_dt")
    nc.sync.dma_start(wt[:], t[:])
    W['dt'] = wt
    return W


def stage4(c):
    """[MT, 4, L] bf16 staging tile (shared tag: h-rhs / patch / outproj)."""
    return c.hp.tile([MT, 4, c.L], BF16, tag="stage4", name="stage4")


def load_h_rhs(c):
    """Stage the full residual as bf16 rhs k-tiles [MT, 4, L]."""
    hb = stage4(c)
    Q = c.L // 4
    for q in range(4):
        c.nc.sync.dma_start(
            hb[:, :, bass.ts(q, Q)],
            c.hfull[:, bass.ts(q, Q)].rearrange("(k m) t -> m k t", k=4))
    return hb


def build_body(c):
    nc = c.nc
    L, nt, jts = c.L, c.nt, c.jts

    # ---- patch embedding: full DIM bf16 rhs + fp32 master for own rows ----
    xr = c.ap.tile([9, L], BF16, tag="xr")
    nc.sync.dma_start(xr[:], c.din['x_rhs'][:])
    hb = stage4(c)
    for m in range(4):
        for j in range(jts):
            ps = c.pp.tile([MT, nt], F32, tag="ps")
            nc.tensor.matmul(ps[:], c.patch_lhsT[:, m * MT:(m + 1) * MT],
                             xr[:, bass.ts(j, nt)], start=True, stop=True)
            nc.scalar.activation(hb[:, m, bass.ts(j, nt)], ps[:], AF.Identity,
                                 bias=c.patch_b[:, m, :])
    for j in range(jts):
        ps = c.pp.tile([MT, nt], F32, tag="ps")
        nc.tensor.matmul(ps[:], c.patch_own_lhsT[:], xr[:, bass.ts(j, nt)],
                         start=True, stop=True)
        nc.scalar.activation(c.h_master[:, bass.ts(j, nt)], ps[:], AF.Identity,
                             bias=c.patch_own_b[:])

    hb = run_block(c, ['in', 'in'], [False, True], hb)
    for i in range(DEPTH):
        hb = run_block(c, [f'l{i}'], [False], hb)
    hb = run_block(c, ['out', 'out'], [False, True], hb)

    # ---- lm head on final residual ----
    for j in range(jts):
        ps = c.pp.tile([MT, nt], F32, tag="ps")
        for k in range(4):
            nc.tensor.matmul(ps[:], c.lm_lhsT[:, k, :], hb[:, k, bass.ts(j, nt)],
                             start=(k == 0), stop=(k == 3))
        ot = c.hp.tile([MT, nt], F32, tag="lmout", bufs=2, name="lmout")
        nc.scalar.activation(ot[:], ps[:], AF.Copy)
        nc.sync.dma_start(c.out_t[:, bass.ts(j, nt)], ot[:])


def run_block(c, sets, revs, hb):
    """One mid layer (sets=[s]) or a bidir pair (sets=[s,s], revs=[F,T]).

    hb: [MT, 4, L] bf16 rhs staging of the current residual. Returns the
    staging tile for the NEXT block (loaded from the post-layer AllGather).
    """
    nc = c.nc
    L, nt, jts = c.L, c.nt, c.jts
    s0 = sets[0]
    ndir = len(sets)
    W = load_weights(c, s0)

    # ---- in_proj (shared between directions; flip commutes w/ pointwise) ----
    xi = [c.ap.tile([CT, L + 6], BF16, tag=f"xi{ct}", name=f"xi{ct}")
          for ct in range(NCT)]
    for ct in range(NCT):
        nc.gpsimd.memset(xi[ct][:, 0:3], 0.0)
        nc.gpsimd.memset(xi[ct][:, 3 + L:], 0.0)
    sz = c.ap.tile([CT, NCT, L], BF16, tag="sz")
    for ct in range(NCT):
        for j in range(jts):
            ps = c.pp.tile([CT, nt], F32, tag="ps")
            for k in range(4):
                nc.tensor.matmul(ps[:], W['wxi'][:, k, ct * CT:(ct + 1) * CT],
                                 hb[:, k, bass.ts(j, nt)],
                                 start=(k == 0), stop=(k == 3))
            nc.scalar.activation(xi[ct][:, 3 + j * nt:3 + (j + 1) * nt], ps[:],
                                 AF.Copy)
            ps2 = c.pp.tile([CT, nt], F32, tag="ps")
            for k in range(4):
                nc.tensor.matmul(ps2[:], W['wz'][:, k, ct * CT:(ct + 1) * CT],
                                 hb[:, k, bass.ts(j, nt)],
                                 start=(k == 0), stop=(k == 3))
            nc.scalar.activation(sz[:, ct, bass.ts(j, nt)], ps2[:], AF.Silu)

    # ---- per-direction conv + xproj partial ----
    xcs = []
    xpst = c.hp.tile([XR, ndir, L], BF16, tag=f"xpst{ndir}", name="xpst")
    for di, (s, rev) in enumerate(zip(sets, revs)):
        xc = c.ap.tile([CT, NCT, L], BF16, tag=f"xc{di}", name=f"xc{di}")
        for ct in range(NCT):
            for j in range(jts):
                ps = c.pp.tile([CT, nt], F32, tag="ps")
                for k in range(KC):
                    off = (6 - k) if rev else k
                    nc.tensor.matmul(ps[:], W['conv'][:, ct, k, :],
                                     xi[ct][:, j * nt + off: j * nt + off + nt],
                                     start=(k == 0), stop=(k == KC - 1))
                nc.scalar.activation(xc[:, ct, bass.ts(j, nt)], ps[:], AF.Silu,
                                     bias=W['convb'][:, ct, :])
        xcs.append(xc)
        for j in range(jts):
            ps = c.pp.tile([XR, nt], F32, tag="psx")
            for ct in range(NCT):
                nc.tensor.matmul(ps[:], W['xp'][:, ct, :],
                                 xc[:, ct, bass.ts(j, nt)],
                                 start=(ct == 0), stop=(ct == NCT - 1))
            nc.scalar.activation(xpst[:, di, bass.ts(j, nt)], ps[:], AF.Copy)
    rows = XR * ndir
    for di in range(ndir):
        nc.sync.dma_start(c.ccin[XR * di:XR * (di + 1), :], xpst[:, di, :])

    # ---- combine x_proj partials within the 4-core group ----
    nc.gpsimd.collective_compute(
        "AllReduce", OP.add, replica_groups=GROUPS,
        ins=[c.ccin[0:rows, :].opt()], outs=[c.ccout[0:rows, :].opt()])

    # ---- per-direction: delta, scan, gating ----
    y2sum = c.ap.tile([CT, NCT, L], BF16, tag="y2sum")
    for di, (s, rev) in enumerate(zip(sets, revs)):
        xc = xcs[di]
        base = XR * di
        dbl30 = c.ap.tile([R, L], BF16, tag="dbl30")
        nc.sync.dma_start(dbl30[:], c.ccout[base:base + R, :])

        delta = c.ap.tile([CT, NCT, L], BF16, tag="delta")
        u = c.ap.tile([CT, NCT, L], BF16, tag="u")
        for ct in range(NCT):
            for j in range(jts):
                ps = c.pp.tile([CT, nt], F32, tag="ps")
                nc.tensor.matmul(ps[:], W['dt'][:, ct * CT:(ct + 1) * CT],
                                 dbl30[:, bass.ts(j, nt)], start=True, stop=True)
                # softplus(x) = ln(1 + e^x)
                spt = c.sp.tile([CT, nt], BF16, tag="spt", bufs=2, name="spt")
                nc.scalar.activation(spt[:], ps[:], AF.Exp,
                                     bias=W['dtb'][:, ct, :])
                nc.scalar.activation(delta[:, ct, bass.ts(j, nt)], spt[:],
                                     AF.Ln, bias=1.0)
            nc.vector.tensor_mul(u[:, ct, :], delta[:, ct, :], xc[:, ct, :])

        bcsrc = c.ccout[base + R:base + R + 2 * NS, :].rearrange(
            "(a n) t -> n a t", a=2)
        for ct in range(NCT):
            ypss = [c.yp.tile([CT, nt], F32, tag=f"y{j}", name=f"yps{j}")
                    for j in range(jts)]
            for n in range(NS):
                ag = c.sp.tile([CT, L], BF16, tag="ag", bufs=5, name=f"ag{n}")
                nc.scalar.activation(ag[:], delta[:, ct, :], AF.Exp,
                                     scale=W['A'][:, ct, n:n + 1])
                brep = c.sp.tile([CT, L], BF16, tag="brep", bufs=3,
                                 name=f"brep{n}")
                nc.sync.dma_start(brep[:], bcsrc[n, 0].partition_broadcast(CT))
                crep = c.sp.tile([CT, L], BF16, tag="crep", bufs=3,
                                 name=f"crep{n}")
                nc.sync.dma_start(crep[:], bcsrc[n, 1].partition_broadcast(CT))
                bg = c.sp.tile([CT, L], BF16, tag="bg", bufs=2, name="bg")
                nc.vector.tensor_mul(bg[:], u[:, ct, :], brep[:])
                hg = c.sp.tile([CT, L], BF16, tag="hg", bufs=3, name="hg")
                if rev:
                    nc.vector.tensor_tensor_scan(hg[:, ::-1], ag[:, ::-1],
                                                 bg[:, ::-1], 0.0,
                                                 OP.mult, OP.add)
                else:
                    nc.vector.tensor_tensor_scan(hg[:], ag[:], bg[:], 0.0,
                                                 OP.mult, OP.add)
                ych = c.sp.tile([CT, L], BF16, tag="ych", bufs=2, name="ych")
                yeng = nc.vector if (n % 2 == 1) else nc.gpsimd
                yeng.tensor_mul(ych[:], hg[:], crep[:])
                for j in range(jts):
                    nc.tensor.matmul(ypss[j][:], c.ident[:],
                                     ych[:, bass.ts(j, nt)],
                                     start=(n == 0), stop=(n == NS - 1))
            for j in range(jts):
                y2p = c.hp.tile([CT, nt], BF16, tag="y2p", bufs=2, name="y2p")
                nc.vector.scalar_tensor_tensor(
                    y2p[:], xc[:, ct, bass.ts(j, nt)], W['D'][:, ct, :],
                    ypss[j][:], op0=OP.mult, op1=OP.add)
                if di == 0:
                    nc.vector.tensor_mul(y2sum[:, ct, bass.ts(j, nt)], y2p[:],
                                         sz[:, ct, bass.ts(j, nt)])
                else:
                    nc.vector.tensor_mul(y2p[:], y2p[:],
                                         sz[:, ct, bass.ts(j, nt)])
                    nc.vector.tensor_add(y2sum[:, ct, bass.ts(j, nt)],
                                         y2sum[:, ct, bass.ts(j, nt)], y2p[:])

    # ---- out_proj partial ----
    opst = stage4(c)
    for m in range(4):
        for j in range(jts):
            ps = c.pp.tile([MT, nt], F32, tag="ps")
            for ct in range(NCT):
                nc.tensor.matmul(ps[:], W['op'][:, ct, m * MT:(m + 1) * MT],
                                 y2sum[:, ct, bass.ts(j, nt)],
                                 start=(ct == 0), stop=(ct == NCT - 1))
            if m % 2 == 0:
                nc.scalar.activation(opst[:, m, bass.ts(j, nt)], ps[:],
                                     AF.Copy)
            else:
                nc.vector.tensor_copy(opst[:, m, bass.ts(j, nt)], ps[:])
        nc.sync.dma_start(c.opin[m * MT:(m + 1) * MT, :], opst[:, m, :])

    # ---- ReduceScatter onto owner's slice; fp32 residual update; AllGather
    nc.gpsimd.collective_compute(
        "ReduceScatter", OP.add, replica_groups=GROUPS,
        ins=[c.opin[:].opt()], outs=[c.rsout[:].opt()])
    rst = c.hp.tile([MT, L], BF16, tag="rst", name="rst")
    nc.sync.dma_start(rst[:], c.rsout[:])
    hbf = c.hp.tile([MT, L], BF16, tag="hbf", name="hbf")
    nc.vector.tensor_add(hbf[:], c.h_master[:], rst[:])
    nc.sync.dma_start(c.agin[:], hbf[:])
    nc.vector.tensor_add(c.h_master[:], c.h_master[:], rst[:])
    nc.gpsimd.collective_compute(
        "AllGather", OP.bypass, replica_groups=GROUPS,
        ins=[c.agin[:].opt()], outs=[c.hfull[:].opt()])
    return load_h_rhs(c)


_KERNEL_CACHE = {}


def get_kernel(L, repeat=1):
    key = (L, repeat)
    if key not in _KERNEL_CACHE:
        _KERNEL_CACHE[key] = build_kernel(L, repeat)
    return _KERNEL_CACHE[key]


def kernel(**inputs):
    L = int(np.asarray(inputs['x']).shape[1])
    nc = get_kernel(L)
    in_maps = [prep_core_inputs(cc, inputs, L) for cc in range(NCORES)]
    res = run_bass_kernel_spmd(nc, in_maps, list(range(NCORES)))
    outs = [np.asarray(res.results[cc]['out'], np.float32) for cc in range(NCORES)]
    full = np.zeros((B, L, V), np.float32)
    for cc in range(NCORES):
        g, rank = divmod(cc, TP)
        full[g, :, rank * MT:(rank + 1) * MT] = outs[cc].T
    return full
